# revision 27
# baseline (speedup 1.0000x reference)
"""Trainium2 Bass kernel for nn_Encoder_38302518345840.

Single-layer transformer encoder (single-head attention with q=k=v=x, then FFN),
B=4, S=2048, D=1024, DFF=4096, fp32.

Sharding: data-parallel over tokens. Core c handles batch c//2, query/token half
c%2 (1024 tokens each). No collectives; each core computes its output slice.

FFN matmuls run in float32r (fp32 with 11-bit mantissa, exact fp32 PSUM
accumulation) at bf16 speed for moving dims >= 256; f32r inputs are pre-rounded
on host and device-side f32r operands are produced with f32r-typed outputs as
the BIR verifier requires. The attention scores and AV matmuls run in bf16:
softmax is invariant to the shared rounding of scores, and the AV rounding of
the dominant diagonal term is cancelled exactly by the residual formulation
t = 2*x_f32r + (attn/r - x_bf16). Softmax skips the max-subtraction (scores/
sqrt(D) <= ~34 here, safely inside fp32/bf16 exp range) since softmax is
shift-invariant when numerator and denominator share the same exponentials.
LayerNorm rsqrt runs on the vector engine (fast-inverse-sqrt seed + 2 Newton
steps, ~1e-6 rel) so ScalarE runs only Exp/Gelu and never thrashes activation
table sets.
"""
import numpy as np
import ml_dtypes

import concourse.bacc as bacc
import concourse.mybir as mybir
import concourse.tile as tile
from concourse.bass_utils import run_bass_kernel_spmd
from concourse.masks import make_identity

dt = mybir.dt
AF = mybir.ActivationFunctionType
ALU = mybir.AluOpType

B, S, D, DFF = 4, 2048, 1024, 4096
EPS = 1e-5
N_CORES = 8
TOK = S // 2          # tokens per core (1024)
QT = 512              # query tile (attention)
TG = 512              # token group (FFN)
SCALE = 1.0 / np.sqrt(D)

KC = S // 128         # 16 key chunks
DC = D // 128         # 8 D chunks
FC = DFF // 128       # 32 dff chunks
NQ = TOK // 128       # 8 token tiles per core
NQT = TOK // QT       # 2 query tiles
NTG = TOK // TG       # 2 token groups


def to_f32r(x: np.ndarray) -> np.ndarray:
    """Round fp32 to fp32r (RNE to 11-bit mantissa; low 12 bits zero)."""
    u = np.ascontiguousarray(x, dtype=np.float32).view(np.uint32).astype(np.uint64)
    lsb = (u >> 12) & 1
    u = (u + 0x7FF + lsb) & np.uint64(0xFFFFF000)
    return u.astype(np.uint32).view(np.float32)


def _emit_rsqrt(nc, pool, var_ap, magic_t, tagp, w=1):
    """rstd = rsqrt(var + eps) on DVE: fast-inverse-sqrt seed + 3 Newton steps.
    Newton step uses tensor_tensor ops so it works for any width w."""
    ve = pool.tile([128, w], dt.float32, tag=f"{tagp}ve", name=f"{tagp}ve")
    nc.vector.tensor_scalar(ve[:], var_ap, EPS, None, op0=ALU.add)
    yi = pool.tile([128, w], dt.int32, tag=f"{tagp}yi", name=f"{tagp}yi")
    nc.vector.tensor_scalar(yi[:], ve[:].bitcast(dt.int32), 1, None,
                            op0=ALU.arith_shift_right)
    y = pool.tile([128, w], dt.float32, tag=f"{tagp}y", name=f"{tagp}y")
    nc.vector.tensor_tensor(y[:].bitcast(dt.int32), magic_t[:, 0:w], yi[:],
                            op=ALU.subtract)
    b = pool.tile([128, w], dt.float32, tag=f"{tagp}b", name=f"{tagp}b")
    c = pool.tile([128, w], dt.float32, tag=f"{tagp}c", name=f"{tagp}c")
    for _ in range(2):
        nc.vector.tensor_tensor(b[:], y[:], y[:], op=ALU.mult)
        nc.vector.tensor_tensor(b[:], b[:], ve[:], op=ALU.mult)
        nc.vector.tensor_scalar(c[:], b[:], -0.5, 1.5, op0=ALU.mult, op1=ALU.add)
        nc.vector.tensor_tensor(y[:], y[:], c[:], op=ALU.mult)
    return y


def build(use_g1: bool, use_g2: bool, use_b2: bool):
    nc = bacc.Bacc("TRN2", target_bir_lowering=False)

    xT = nc.dram_tensor("xT", [128, DC, S], dt.bfloat16, kind="ExternalInput")
    xkd = nc.dram_tensor("xkd", [128, KC, D], dt.bfloat16, kind="ExternalInput")
    xres = nc.dram_tensor("xres", [128, NQ, D], dt.float32r, kind="ExternalInput")
    maskT = nc.dram_tensor("maskT", [128, KC], dt.float32, kind="ExternalInput")
    w1T = nc.dram_tensor("w1T", [128, DC, DFF], dt.float32r, kind="ExternalInput")
    w2T = nc.dram_tensor("w2T", [128, FC, D], dt.float32r, kind="ExternalInput")
    b1T = nc.dram_tensor("b1T", [128, FC], dt.float32, kind="ExternalInput")
    out = nc.dram_tensor("out", [128, NQ, D], dt.float32, kind="ExternalOutput")
    if use_g1:
        g1b = nc.dram_tensor("g1b", [128, D], dt.float32, kind="ExternalInput")
        b1b = nc.dram_tensor("b1b", [128, D], dt.float32, kind="ExternalInput")
    if use_g2:
        g2b = nc.dram_tensor("g2b", [128, D], dt.float32, kind="ExternalInput")
        b2lb = nc.dram_tensor("b2lb", [128, D], dt.float32, kind="ExternalInput")
    if use_b2:
        b2b = nc.dram_tensor("b2b", [128, D], dt.float32, kind="ExternalInput")

    with tile.TileContext(nc) as tc:
        with tc.tile_pool(name="persist", bufs=1) as pp:
            # small constants
            ident_f = pp.tile([128, 128], dt.float32)
            make_identity(nc, ident_f[:])
            ident_r = pp.tile([128, 128], dt.float32r)
            nc.vector.tensor_copy(ident_r[:], ident_f[:])
            ones_f = pp.tile([128, 1], dt.float32)
            nc.vector.memset(ones_f[:], 1.0)
            eps_t = pp.tile([128, 1], dt.float32)
            nc.vector.memset(eps_t[:], EPS)
            magic_t = pp.tile([128, 4], dt.int32)
            nc.vector.memset(magic_t[:], 0x5F3759DF)
            mask_t = pp.tile([128, KC], dt.float32)
            nc.sync.dma_start(mask_t[:], maskT[:])
            b1_t = pp.tile([128, FC], dt.float32)
            nc.gpsimd.dma_start(b1_t[:], b1T[:])
            gb_tiles = {}
            for flag, names in ((use_g1, ("g1b", "b1b")), (use_g2, ("g2b", "b2lb")),
                                (use_b2, ("b2b",))):
                if flag:
                    for nm_ in names:
                        t = pp.tile([128, D], dt.float32, tag=nm_)
                        nc.sync.dma_start(t[:], {"g1b": g1b, "b1b": b1b, "g2b": g2b,
                                                 "b2lb": b2lb, "b2b": b2b}[nm_][:])
                        gb_tiles[nm_] = t

            h_tiles = [pp.tile([128, D], dt.float32r, tag=f"h{i}", name=f"h_{i}")
                       for i in range(NQ)]

            # ---------------- Phase A: attention + LN1 -> h_all ----------------
            with tc.tile_pool(name="a_sb", bufs=1) as asb, \
                 tc.tile_pool(name="a_pT", bufs=1) as apT, \
                 tc.tile_pool(name="a_pr", bufs=1) as apr, \
                 tc.tile_pool(name="a_xtq", bufs=2) as axq, \
                 tc.tile_pool(name="a_xtk", bufs=2) as axk, \
                 tc.tile_pool(name="a_scr", bufs=2) as ascr, \
                 tc.tile_pool(name="a_st", bufs=4) as ast, \
                 tc.tile_pool(name="ps_s", bufs=3, space="PSUM") as ps_s, \
                 tc.tile_pool(name="ps_a", bufs=2, space="PSUM") as ps_a, \
                 tc.tile_pool(name="ps_r", bufs=1, space="PSUM") as ps_r:

                xkd_t = asb.tile([128, KC, D], dt.bfloat16)
                xres_t = asb.tile([128, NQ, D], dt.float32r)

                for qt in range(NQT):
                    q0 = qt * QT  # local token offset of this query tile
                    # ---- scores^T + exp -> pT (f32r) ----
                    xq_t = axq.tile([128, DC, QT], dt.bfloat16, tag="xq")
                    nc.sync.dma_start(xq_t[:, 0:4, :], xT[:, 0:4, q0:q0 + QT])
                    pT = apT.tile([128, KC, QT], dt.bfloat16, tag="pT", name=f"pT{qt}")
                    prps = []
                    for kc4 in range(KC // 4):
                        xk_t = axk.tile([128, DC, 512], dt.bfloat16, tag="xk")
                        ksl = slice(kc4 * 512, (kc4 + 1) * 512)
                        nc.sync.dma_start(xk_t[:, 0:4, :], xT[:, 0:4, ksl])
                        if kc4 == 0:
                            nc.sync.dma_start(xq_t[:, 4:8, :],
                                              xT[:, 4:8, q0:q0 + QT])
                        nc.sync.dma_start(xk_t[:, 4:8, :], xT[:, 4:8, ksl])
                        if qt == 0:
                            for j in range(4):
                                c = kc4 * 4 + j
                                nc.sync.dma_start(xkd_t[:, c, :], xkd[:, c, :])
                            nc.sync.dma_start(xres_t[:, kc4 * 2:kc4 * 2 + 2, :],
                                              xres[:, kc4 * 2:kc4 * 2 + 2, :])
                        for kk in range(4):
                            kc = kc4 * 4 + kk
                            s_ps = ps_s.tile([128, QT], dt.float32, tag="s")
                            for dc in range(DC):
                                nc.tensor.matmul(s_ps[:],
                                                 xk_t[:, dc, kk * 128:(kk + 1) * 128],
                                                 xq_t[:, dc, :],
                                                 start=(dc == 0), stop=(dc == DC - 1))
                            nc.scalar.activation(pT[:, kc, :], s_ps[:], AF.Exp,
                                                 bias=mask_t[:, kc:kc + 1], scale=SCALE)
                        prp = apr.tile([128, QT], dt.float32, tag=f"prp{kc4}",
                                       name=f"prp_{qt}_{kc4}")
                        nc.vector.tensor_reduce(
                            prp[:],
                            pT[:, kc4 * 4:(kc4 + 1) * 4, :]
                            .rearrange("p k q -> p q k"),
                            op=ALU.add, axis=mybir.AxisListType.X)
                        prps.append(prp)
                    # ---- combine partial row-sums (tree) ----
                    pr = ascr.tile([128, QT], dt.float32, tag="pr")
                    pr2 = ascr.tile([128, QT], dt.float32, tag="pr2")
                    nc.vector.tensor_add(pr2[:], prps[0][:], prps[1][:])
                    nc.vector.tensor_add(pr[:], prps[2][:], prps[3][:])
                    nc.vector.tensor_add(pr[:], pr[:], pr2[:])
                    # ---- AV + r + LN1 per 128-token tile ----
                    for qi4 in range(QT // 128):
                        qi = qt * (QT // 128) + qi4  # local token tile index (0..7)
                        a_ps = ps_a.tile([128, D], dt.float32, tag="attn")
                        r_ps = ps_r.tile([128, 1], dt.float32, tag="r")
                        for kc in range(KC):
                            lhsT = pT[:, kc, qi4 * 128:(qi4 + 1) * 128]
                            st, sp = (kc == 0), (kc == KC - 1)
                            nc.tensor.matmul(a_ps[:, 0:512], lhsT,
                                             xkd_t[:, kc, 0:512], start=st, stop=sp)
                            nc.tensor.matmul(a_ps[:, 512:1024], lhsT,
                                             xkd_t[:, kc, 512:1024], start=st, stop=sp)
                        nc.tensor.matmul(r_ps[:],
                                         pr[:, qi4 * 128:(qi4 + 1) * 128],
                                         ones_f[:], start=True, stop=True)
                        r_inv = ast.tile([128, 1], dt.float32, tag="rinv")
                        nc.vector.reciprocal(r_inv[:], r_ps[:])
                        t_sb = ascr.tile([128, 2, 512], dt.float32, tag="t1")
                        u_sb = ascr.tile([128, D], dt.float32, tag="u1")
                        # chunk qi of xkd is this core's token tile qi (host reorders).
                        # t = 2*x_f32r + (attn/r - x_bf16): the bf16 rounding of the
                        # dominant (diagonal) attention term cancels exactly.
                        nc.vector.scalar_tensor_tensor(
                            u_sb[:], a_ps[:], r_inv[:], xkd_t[:, qi, :],
                            op0=ALU.mult, op1=ALU.subtract)
                        nc.vector.scalar_tensor_tensor(
                            t_sb[:].rearrange("p a b -> p (a b)"),
                            xres_t[:, qi, :].bitcast(dt.float32), 2.0, u_sb[:],
                            op0=ALU.mult, op1=ALU.add)
                        bn6 = ast.tile([128, 2, 6], dt.float32, tag="bn6")
                        nc.vector.bn_stats(bn6[:, 0, :], t_sb[:, 0, :])
                        nc.vector.bn_stats(bn6[:, 1, :], t_sb[:, 1, :])
                        bn2 = ast.tile([128, 2], dt.float32, tag="bn2")
                        nc.vector.bn_aggr(bn2[:], bn6[:])
                        rstd = _emit_rsqrt(nc, ast, bn2[:, 1:2], magic_t, "a")
                        nm = ast.tile([128, 1], dt.float32, tag="nm")
                        nc.vector.tensor_scalar(nm[:], bn2[:, 0:1], rstd[:], -1.0,
                                                op0=ALU.mult, op1=ALU.mult)
                        tv = t_sb[:].rearrange("p a b -> p (a b)")
                        if use_g1:
                            hn = ascr.tile([128, D], dt.float32, tag="hn")
                            nc.scalar.activation(hn[:], tv, AF.Identity,
                                                 bias=nm[:], scale=rstd[:])
                            hg = ascr.tile([128, D], dt.float32, tag="hg")
                            nc.vector.tensor_mul(hg[:], hn[:], gb_tiles["g1b"][:])
                            nc.vector.tensor_add(h_tiles[qi][:], hg[:],
                                                 gb_tiles["b1b"][:])
                        else:
                            nc.vector.tensor_scalar(h_tiles[qi][:], tv, rstd[:],
                                                    nm[:], op0=ALU.mult, op1=ALU.add)

            # ---------------- Phase B: transpose h, FFN, LN2 -> out ----------------
            with tc.tile_pool(name="b_sb", bufs=1) as bsb, \
                 tc.tile_pool(name="b_w1", bufs=3) as bw1, \
                 tc.tile_pool(name="b_w2", bufs=3) as bw2, \
                 tc.tile_pool(name="b_out", bufs=3) as bout, \
                 tc.tile_pool(name="b_st", bufs=4) as bst, \
                 tc.tile_pool(name="b_t2", bufs=4) as bt2, \
                 tc.tile_pool(name="b_scr", bufs=2) as bscr, \
                 tc.tile_pool(name="ps_tr", bufs=2, space="PSUM") as ps_tr, \
                 tc.tile_pool(name="ps_z", bufs=4, space="PSUM") as ps_z, \
                 tc.tile_pool(name="ps_y", bufs=2, space="PSUM") as ps_y:

                hT = bsb.tile([128, DC, TOK], dt.float32r)

                def emit_transposes(qlo_, qhi_):
                    # half-tile pipelining: 2 PSUM slots, evac alternates ACT/DVE
                    for qi in range(qlo_, qhi_):
                        for hf in range(2):
                            tr_ps = ps_tr.tile([128, 512], dt.float32r, tag="tr",
                                               name=f"tr_{qi}_{hf}")
                            for j in range(4):
                                dc = hf * 4 + j
                                nc.tensor.transpose(
                                    tr_ps[:, j * 128:(j + 1) * 128],
                                    h_tiles[qi][:, dc * 128:(dc + 1) * 128],
                                    ident_r[:])
                            dst = hT[:, hf * 4:(hf + 1) * 4,
                                     qi * 128:(qi + 1) * 128]
                            src = tr_ps[:].rearrange("p (c t) -> p c t", c=4)
                            if (qi + hf) % 2 == 0:
                                nc.scalar.copy(dst, src)
                            else:
                                nc.vector.tensor_copy(dst, src)

                for tg in range(NTG):
                    emit_transposes(tg * 4, (tg + 1) * 4)
                    t0 = tg * TG
                    nt = TG // 128  # 4 token tiles in this group
                    # ---- stage A: gT = gelu(w1^T @ hT + b1) ----
                    gT = bsb.tile([128, FC, TG], dt.float32r, tag="gT")
                    for fc4 in range(FC // 2):
                        w1_t = bw1.tile([128, DC, 256], dt.float32r, tag="w1")
                        nc.sync.dma_start(w1_t[:, 0:4, :],
                                          w1T[:, 0:4, fc4 * 256:(fc4 + 1) * 256])
                        nc.sync.dma_start(w1_t[:, 4:8, :],
                                          w1T[:, 4:8, fc4 * 256:(fc4 + 1) * 256])
                        for kk in range(2):
                            fc = fc4 * 2 + kk
                            y_ps = ps_y.tile([128, TG], dt.float32, tag="y")
                            for dc in range(DC):
                                nc.tensor.matmul(y_ps[:],
                                                 w1_t[:, dc, kk * 128:(kk + 1) * 128],
                                                 hT[:, dc, t0:t0 + TG],
                                                 start=(dc == 0), stop=(dc == DC - 1))
                            nc.scalar.activation(gT[:, fc, :], y_ps[:], AF.Gelu,
                                                 bias=b1_t[:, fc:fc + 1])
                    # ---- stage B: z = gT^T @ w2 per D-half; assemble t2 = z + h ----
                    t2s = [bt2.tile([128, 2, 512], dt.float32, tag="t2",
                                    name=f"t2_{tg}_{i}")
                           for i in range(nt)]
                    bn64 = bst.tile([128, nt, 2, 6], dt.float32, tag="bn64")
                    bn24 = bst.tile([128, nt, 2], dt.float32, tag="bn24")
                    for dh in range(2):
                        dsl = slice(dh * 512, (dh + 1) * 512)
                        z_ps = [ps_z.tile([128, 512], dt.float32, tag="z",
                                          name=f"z_{tg}_{dh}_{i}")
                                for i in range(nt)]
                        for fc4 in range(FC // 4):
                            w2_t = bw2.tile([128, 4, 512], dt.float32r, tag="w2")
                            nc.sync.dma_start(w2_t[:],
                                              w2T[:, fc4 * 4:(fc4 + 1) * 4, dsl])
                            for kk in range(4):
                                fc = fc4 * 4 + kk
                                for tt in range(nt):
                                    nc.tensor.matmul(
                                        z_ps[tt][:],
                                        gT[:, fc, tt * 128:(tt + 1) * 128],
                                        w2_t[:, kk, :],
                                        start=(fc == 0), stop=(fc == FC - 1))
                        for tt in range(nt):
                            qi = tg * nt + tt
                            nc.vector.tensor_add(
                                t2s[tt][:, dh, :], z_ps[tt][:],
                                h_tiles[qi][:, dsl].bitcast(dt.float32))
                            if use_b2:
                                nc.vector.tensor_add(t2s[tt][:, dh, :],
                                                     t2s[tt][:, dh, :],
                                                     gb_tiles["b2b"][:, dsl])
                            nc.vector.bn_stats(bn64[:, tt, dh, :], t2s[tt][:, dh, :])
                    # ---- LN2 (stats vectorized over the nt tiles) + store ----
                    for tt in range(nt):
                        nc.vector.bn_aggr(bn24[:, tt, :], bn64[:, tt, :, :])
                    rstd4 = _emit_rsqrt(nc, bst, bn24[:, :, 1], magic_t, "b", w=nt)
                    nm4 = bst.tile([128, nt], dt.float32, tag="nm4")
                    nc.vector.scalar_tensor_tensor(nm4[:], bn24[:, :, 0], -1.0,
                                                   rstd4[:], op0=ALU.mult,
                                                   op1=ALU.mult)
                    for tt in range(nt):
                        qi = tg * nt + tt
                        t2v = t2s[tt][:].rearrange("p a b -> p (a b)")
                        o_sb = bout.tile([128, D], dt.float32, tag="osb")
                        if use_g2:
                            on = bscr.tile([128, D], dt.float32, tag="on")
                            nc.scalar.activation(on[:], t2v, AF.Identity,
                                                 bias=nm4[:, tt:tt + 1],
                                                 scale=rstd4[:, tt:tt + 1])
                            og = bscr.tile([128, D], dt.float32, tag="og")
                            nc.vector.tensor_mul(og[:], on[:], gb_tiles["g2b"][:])
                            nc.vector.tensor_add(o_sb[:], og[:], gb_tiles["b2lb"][:])
                        else:
                            nc.scalar.activation(o_sb[:], t2v, AF.Identity,
                                                 bias=nm4[:, tt:tt + 1],
                                                 scale=rstd4[:, tt:tt + 1])
                        nc.sync.dma_start(out[:, qi, :], o_sb[:])

    nc.finalize()
    return nc


_BUILD_CACHE = {}


def kernel(hidden_state, attention_mask, w1, b1, w2, b2,
           ln1_g, ln1_b, ln2_g, ln2_b) -> np.ndarray:
    hidden_state = np.asarray(hidden_state, dtype=np.float32)
    attention_mask = np.asarray(attention_mask, dtype=np.float32)
    w1 = np.asarray(w1, dtype=np.float32)
    b1 = np.asarray(b1, dtype=np.float32)
    w2 = np.asarray(w2, dtype=np.float32)
    b2 = np.asarray(b2, dtype=np.float32)
    ln1_g = np.asarray(ln1_g, dtype=np.float32)
    ln1_b = np.asarray(ln1_b, dtype=np.float32)
    ln2_g = np.asarray(ln2_g, dtype=np.float32)
    ln2_b = np.asarray(ln2_b, dtype=np.float32)

    use_g1 = not (np.all(ln1_g == 1.0) and np.all(ln1_b == 0.0))
    use_g2 = not (np.all(ln2_g == 1.0) and np.all(ln2_b == 0.0))
    use_b2 = bool(np.any(b2 != 0.0))

    key = (use_g1, use_g2, use_b2)
    if key not in _BUILD_CACHE:
        _BUILD_CACHE[key] = build(*key)
    nc = _BUILD_CACHE[key]

    # host-side layout prep (shared across cores of the same batch)
    w1_l = to_f32r(w1.reshape(DC, 128, DFF).transpose(1, 0, 2))     # [128, DC, DFF]
    w2_l = to_f32r(w2.reshape(FC, 128, D).transpose(1, 0, 2))       # [128, FC, D]
    b1_l = np.ascontiguousarray(b1.reshape(FC, 128).T)              # [128, FC]

    in_maps = []
    for c in range(N_CORES):
        b_idx, half = c // 2, c % 2
        x = hidden_state[b_idx]                                     # [S, D]
        xr = to_f32r(x)
        xT_l = np.ascontiguousarray(
            x.astype(ml_dtypes.bfloat16).T.reshape(DC, 128, S)
            .transpose(1, 0, 2))                                    # [128, DC, S] bf16
        xkd_f = xr.reshape(KC, 128, D).transpose(1, 0, 2)           # [128, KC, D] f32r
        # NOTE: xkd rows used for residual must be this core's token rows:
        # token tile qi (0..7) = batch row chunk half*8 + qi. The kernel indexes
        # xkd_t[:, qi, :] for the residual, so reorder chunks: put this core's
        # 8 chunks first, the other half after (AV sweeps all 16 either way,
        # with the exp/mask/pT ordering matching the chunk order).
        order = list(range(half * 8, half * 8 + 8)) + \
                list(range((1 - half) * 8, (1 - half) * 8 + 8))
        xkd_f = xkd_f[:, order, :]
        xres_l = np.ascontiguousarray(xkd_f[:, 0:NQ, :])            # this core's rows
        xkd_l = np.ascontiguousarray(
            x.astype(ml_dtypes.bfloat16).reshape(KC, 128, D)
            .transpose(1, 0, 2)[:, order, :])                       # [128, KC, D] bf16
        mask_l = np.ascontiguousarray(
            attention_mask[b_idx, 0].reshape(KC, 128).T)            # [128, KC]
        mask_l = np.ascontiguousarray(mask_l[:, order])
        # xT column order must match xkd chunk order for the scores (keys) too:
        kcols = np.concatenate([np.arange(o * 128, (o + 1) * 128) for o in order])
        xT_l = np.ascontiguousarray(xT_l[:, :, kcols])
        # queries: this core's tokens are the FIRST 8 chunks in reordered space
        im = dict(xT=xT_l, xkd=xkd_l, xres=xres_l, maskT=mask_l, w1T=w1_l,
          w2T=w2_l, b1T=b1_l)
        if use_g1:
            im["g1b"] = np.ascontiguousarray(np.broadcast_to(ln1_g, (128, D)))
            im["b1b"] = np.ascontiguousarray(np.broadcast_to(ln1_b, (128, D)))
        if use_g2:
            im["g2b"] = np.ascontiguousarray(np.broadcast_to(ln2_g, (128, D)))
            im["b2lb"] = np.ascontiguousarray(np.broadcast_to(ln2_b, (128, D)))
        if use_b2:
            im["b2b"] = np.ascontiguousarray(np.broadcast_to(b2, (128, D)))
        in_maps.append(im)

    res = run_bass_kernel_spmd(nc, in_maps, core_ids=list(range(N_CORES)))

    out_full = np.empty((B, S, D), dtype=np.float32)
    for c in range(N_CORES):
        b_idx, half = c // 2, c % 2
        o = res.results[c]["out"]                                   # [128, NQ, D]
        rows = o.transpose(1, 0, 2).reshape(TOK, D)
        out_full[b_idx, half * TOK:(half + 1) * TOK] = rows
    return out_full


# revision 28
# speedup vs baseline: 1.0182x; 1.0182x over previous
"""Trainium2 Bass kernel for nn_Encoder_38302518345840.

Single-layer transformer encoder (single-head attention with q=k=v=x, then FFN),
B=4, S=2048, D=1024, DFF=4096, fp32.

Sharding: data-parallel over tokens. Core c handles batch c//2, query/token half
c%2 (1024 tokens each). No collectives; each core computes its output slice.

FFN matmuls run in float32r (fp32 with 11-bit mantissa, exact fp32 PSUM
accumulation) at bf16 speed for moving dims >= 256; f32r inputs are pre-rounded
on host and device-side f32r operands are produced with f32r-typed outputs as
the BIR verifier requires. The attention scores and AV matmuls run in bf16:
softmax is invariant to the shared rounding of scores, and the AV rounding of
the dominant diagonal term is cancelled exactly by the residual formulation
t = 2*x_f32r + (attn/r - x_bf16). Softmax skips the max-subtraction (scores/
sqrt(D) <= ~34 here, safely inside fp32/bf16 exp range) since softmax is
shift-invariant when numerator and denominator share the same exponentials.
LayerNorm rsqrt runs on the vector engine (fast-inverse-sqrt seed + 2 Newton
steps, ~1e-6 rel) so ScalarE runs only Exp/Gelu and never thrashes activation
table sets.
"""
import numpy as np
import ml_dtypes

import concourse.bacc as bacc
import concourse.mybir as mybir
import concourse.tile as tile
from concourse.bass_utils import run_bass_kernel_spmd
from concourse.masks import make_identity

dt = mybir.dt
AF = mybir.ActivationFunctionType
ALU = mybir.AluOpType

B, S, D, DFF = 4, 2048, 1024, 4096
EPS = 1e-5
N_CORES = 8
TOK = S // 2          # tokens per core (1024)
QT = 512              # query tile (attention)
TG = 512              # token group (FFN)
SCALE = 1.0 / np.sqrt(D)

KC = S // 128         # 16 key chunks
DC = D // 128         # 8 D chunks
FC = DFF // 128       # 32 dff chunks
NQ = TOK // 128       # 8 token tiles per core
NQT = TOK // QT       # 2 query tiles
NTG = TOK // TG       # 2 token groups


def to_f32r(x: np.ndarray) -> np.ndarray:
    """Round fp32 to fp32r (RNE to 11-bit mantissa; low 12 bits zero)."""
    u = np.ascontiguousarray(x, dtype=np.float32).view(np.uint32).astype(np.uint64)
    lsb = (u >> 12) & 1
    u = (u + 0x7FF + lsb) & np.uint64(0xFFFFF000)
    return u.astype(np.uint32).view(np.float32)


def _emit_rsqrt(nc, pool, var_ap, magic_t, tagp, w=1):
    """rstd = rsqrt(var + eps) on DVE: fast-inverse-sqrt seed + 3 Newton steps.
    Newton step uses tensor_tensor ops so it works for any width w."""
    ve = pool.tile([128, w], dt.float32, tag=f"{tagp}ve", name=f"{tagp}ve")
    nc.vector.tensor_scalar(ve[:], var_ap, EPS, None, op0=ALU.add)
    yi = pool.tile([128, w], dt.int32, tag=f"{tagp}yi", name=f"{tagp}yi")
    nc.vector.tensor_scalar(yi[:], ve[:].bitcast(dt.int32), 1, None,
                            op0=ALU.arith_shift_right)
    y = pool.tile([128, w], dt.float32, tag=f"{tagp}y", name=f"{tagp}y")
    nc.vector.tensor_tensor(y[:].bitcast(dt.int32), magic_t[:, 0:w], yi[:],
                            op=ALU.subtract)
    b = pool.tile([128, w], dt.float32, tag=f"{tagp}b", name=f"{tagp}b")
    c = pool.tile([128, w], dt.float32, tag=f"{tagp}c", name=f"{tagp}c")
    for _ in range(2):
        nc.vector.tensor_tensor(b[:], y[:], y[:], op=ALU.mult)
        nc.vector.tensor_tensor(b[:], b[:], ve[:], op=ALU.mult)
        nc.vector.tensor_scalar(c[:], b[:], -0.5, 1.5, op0=ALU.mult, op1=ALU.add)
        nc.vector.tensor_tensor(y[:], y[:], c[:], op=ALU.mult)
    return y


def build(use_g1: bool, use_g2: bool, use_b2: bool):
    nc = bacc.Bacc("TRN2", target_bir_lowering=False)

    xT = nc.dram_tensor("xT", [128, DC, S], dt.bfloat16, kind="ExternalInput")
    xkd = nc.dram_tensor("xkd", [128, KC, D], dt.bfloat16, kind="ExternalInput")
    xres = nc.dram_tensor("xres", [128, NQ, D], dt.float32r, kind="ExternalInput")
    maskT = nc.dram_tensor("maskT", [128, KC], dt.float32, kind="ExternalInput")
    w1T = nc.dram_tensor("w1T", [128, DC, DFF], dt.float32r, kind="ExternalInput")
    w2T = nc.dram_tensor("w2T", [128, FC, D], dt.float32r, kind="ExternalInput")
    b1T = nc.dram_tensor("b1T", [128, FC], dt.float32, kind="ExternalInput")
    out = nc.dram_tensor("out", [128, NQ, D], dt.float32, kind="ExternalOutput")
    if use_g1:
        g1b = nc.dram_tensor("g1b", [128, D], dt.float32, kind="ExternalInput")
        b1b = nc.dram_tensor("b1b", [128, D], dt.float32, kind="ExternalInput")
    if use_g2:
        g2b = nc.dram_tensor("g2b", [128, D], dt.float32, kind="ExternalInput")
        b2lb = nc.dram_tensor("b2lb", [128, D], dt.float32, kind="ExternalInput")
    if use_b2:
        b2b = nc.dram_tensor("b2b", [128, D], dt.float32, kind="ExternalInput")

    with tile.TileContext(nc) as tc:
        with tc.tile_pool(name="persist", bufs=1) as pp:
            # small constants
            ident_f = pp.tile([128, 128], dt.float32)
            make_identity(nc, ident_f[:])
            ident_r = pp.tile([128, 128], dt.float32r)
            nc.vector.tensor_copy(ident_r[:], ident_f[:])
            ones_f = pp.tile([128, 1], dt.float32)
            nc.vector.memset(ones_f[:], 1.0)
            eps_t = pp.tile([128, 1], dt.float32)
            nc.vector.memset(eps_t[:], EPS)
            magic_t = pp.tile([128, 4], dt.int32)
            nc.vector.memset(magic_t[:], 0x5F3759DF)
            mask_t = pp.tile([128, KC], dt.float32)
            nc.sync.dma_start(mask_t[:], maskT[:])
            b1_t = pp.tile([128, FC], dt.float32)
            nc.gpsimd.dma_start(b1_t[:], b1T[:])
            gb_tiles = {}
            for flag, names in ((use_g1, ("g1b", "b1b")), (use_g2, ("g2b", "b2lb")),
                                (use_b2, ("b2b",))):
                if flag:
                    for nm_ in names:
                        t = pp.tile([128, D], dt.float32, tag=nm_)
                        nc.sync.dma_start(t[:], {"g1b": g1b, "b1b": b1b, "g2b": g2b,
                                                 "b2lb": b2lb, "b2b": b2b}[nm_][:])
                        gb_tiles[nm_] = t

            h_tiles = [pp.tile([128, D], dt.float32r, tag=f"h{i}", name=f"h_{i}")
                       for i in range(NQ)]

            # ---------------- Phase A: attention + LN1 -> h_all ----------------
            with tc.tile_pool(name="a_sb", bufs=1) as asb, \
                 tc.tile_pool(name="a_pT", bufs=1) as apT, \
                 tc.tile_pool(name="a_pr", bufs=1) as apr, \
                 tc.tile_pool(name="a_xtq", bufs=2) as axq, \
                 tc.tile_pool(name="a_xtk", bufs=2) as axk, \
                 tc.tile_pool(name="a_scr", bufs=2) as ascr, \
                 tc.tile_pool(name="a_st", bufs=4) as ast, \
                 tc.tile_pool(name="ps_s", bufs=3, space="PSUM") as ps_s, \
                 tc.tile_pool(name="ps_a", bufs=2, space="PSUM") as ps_a, \
                 tc.tile_pool(name="ps_r", bufs=1, space="PSUM") as ps_r:

                xkd_t = asb.tile([128, KC, D], dt.bfloat16)
                xres_t = asb.tile([128, NQ, D], dt.float32r)

                for qt in range(NQT):
                    q0 = qt * QT  # local token offset of this query tile
                    # ---- scores^T + exp -> pT (f32r) ----
                    xq_t = axq.tile([128, DC, QT], dt.bfloat16, tag="xq")
                    nc.sync.dma_start(xq_t[:, 0:4, :], xT[:, 0:4, q0:q0 + QT])
                    pT = apT.tile([128, KC, QT], dt.bfloat16, tag="pT", name=f"pT{qt}")
                    prps = []
                    for kc4 in range(KC // 4):
                        xk_t = axk.tile([128, DC, 512], dt.bfloat16, tag="xk")
                        ksl = slice(kc4 * 512, (kc4 + 1) * 512)
                        nc.sync.dma_start(xk_t[:, 0:4, :], xT[:, 0:4, ksl])
                        if kc4 == 0:
                            nc.sync.dma_start(xq_t[:, 4:8, :],
                                              xT[:, 4:8, q0:q0 + QT])
                        nc.sync.dma_start(xk_t[:, 4:8, :], xT[:, 4:8, ksl])
                        if qt == 0:
                            for j in range(4):
                                c = kc4 * 4 + j
                                nc.sync.dma_start(xkd_t[:, c, :], xkd[:, c, :])
                            nc.sync.dma_start(xres_t[:, kc4 * 2:kc4 * 2 + 2, :],
                                              xres[:, kc4 * 2:kc4 * 2 + 2, :])
                        for kk in range(4):
                            kc = kc4 * 4 + kk
                            s_ps = ps_s.tile([128, QT], dt.float32, tag="s")
                            for dc in range(DC):
                                nc.tensor.matmul(s_ps[:],
                                                 xk_t[:, dc, kk * 128:(kk + 1) * 128],
                                                 xq_t[:, dc, :],
                                                 start=(dc == 0), stop=(dc == DC - 1))
                            nc.scalar.activation(pT[:, kc, :], s_ps[:], AF.Exp,
                                                 bias=mask_t[:, kc:kc + 1], scale=SCALE)
                        prp = apr.tile([128, QT], dt.float32, tag=f"prp{kc4}",
                                       name=f"prp_{qt}_{kc4}")
                        nc.vector.tensor_reduce(
                            prp[:],
                            pT[:, kc4 * 4:(kc4 + 1) * 4, :]
                            .rearrange("p k q -> p q k"),
                            op=ALU.add, axis=mybir.AxisListType.X)
                        prps.append(prp)
                    # ---- combine partial row-sums (tree) ----
                    pr = ascr.tile([128, QT], dt.float32, tag="pr")
                    pr2 = ascr.tile([128, QT], dt.float32, tag="pr2")
                    nc.vector.tensor_add(pr2[:], prps[0][:], prps[1][:])
                    nc.vector.tensor_add(pr[:], prps[2][:], prps[3][:])
                    nc.vector.tensor_add(pr[:], pr[:], pr2[:])
                    # ---- AV + r + LN1 per 128-token tile ----
                    for qi4 in range(QT // 128):
                        qi = qt * (QT // 128) + qi4  # local token tile index (0..7)
                        a_ps = ps_a.tile([128, D], dt.float32, tag="attn")
                        r_ps = ps_r.tile([128, 1], dt.float32, tag="r")
                        for kc in range(KC):
                            lhsT = pT[:, kc, qi4 * 128:(qi4 + 1) * 128]
                            st, sp = (kc == 0), (kc == KC - 1)
                            nc.tensor.matmul(a_ps[:, 0:512], lhsT,
                                             xkd_t[:, kc, 0:512], start=st, stop=sp)
                            nc.tensor.matmul(a_ps[:, 512:1024], lhsT,
                                             xkd_t[:, kc, 512:1024], start=st, stop=sp)
                        nc.tensor.matmul(r_ps[:],
                                         pr[:, qi4 * 128:(qi4 + 1) * 128],
                                         ones_f[:], start=True, stop=True)
                        r_inv = ast.tile([128, 1], dt.float32, tag="rinv")
                        nc.vector.reciprocal(r_inv[:], r_ps[:])
                        t_sb = ascr.tile([128, 2, 512], dt.float32, tag="t1")
                        u_sb = ascr.tile([128, D], dt.float32, tag="u1")
                        # chunk qi of xkd is this core's token tile qi (host reorders).
                        # t = 2*x_f32r + (attn/r - x_bf16): the bf16 rounding of the
                        # dominant (diagonal) attention term cancels exactly.
                        nc.vector.scalar_tensor_tensor(
                            u_sb[:], a_ps[:], r_inv[:], xkd_t[:, qi, :],
                            op0=ALU.mult, op1=ALU.subtract)
                        nc.vector.scalar_tensor_tensor(
                            t_sb[:].rearrange("p a b -> p (a b)"),
                            xres_t[:, qi, :].bitcast(dt.float32), 2.0, u_sb[:],
                            op0=ALU.mult, op1=ALU.add)
                        bn6 = ast.tile([128, 2, 6], dt.float32, tag="bn6")
                        nc.vector.bn_stats(bn6[:, 0, :], t_sb[:, 0, :])
                        nc.vector.bn_stats(bn6[:, 1, :], t_sb[:, 1, :])
                        bn2 = ast.tile([128, 2], dt.float32, tag="bn2")
                        nc.vector.bn_aggr(bn2[:], bn6[:])
                        rstd = _emit_rsqrt(nc, ast, bn2[:, 1:2], magic_t, "a")
                        nm = ast.tile([128, 1], dt.float32, tag="nm")
                        nc.vector.tensor_scalar(nm[:], bn2[:, 0:1], rstd[:], -1.0,
                                                op0=ALU.mult, op1=ALU.mult)
                        tv = t_sb[:].rearrange("p a b -> p (a b)")
                        if use_g1:
                            hn = ascr.tile([128, D], dt.float32, tag="hn")
                            nc.scalar.activation(hn[:], tv, AF.Identity,
                                                 bias=nm[:], scale=rstd[:])
                            hg = ascr.tile([128, D], dt.float32, tag="hg")
                            nc.vector.tensor_mul(hg[:], hn[:], gb_tiles["g1b"][:])
                            nc.vector.tensor_add(h_tiles[qi][:], hg[:],
                                                 gb_tiles["b1b"][:])
                        else:
                            nc.vector.tensor_scalar(h_tiles[qi][:], tv, rstd[:],
                                                    nm[:], op0=ALU.mult, op1=ALU.add)

            # ---------------- Phase B: transpose h, FFN, LN2 -> out ----------------
            with tc.tile_pool(name="b_sb", bufs=1) as bsb, \
                 tc.tile_pool(name="b_w1", bufs=3) as bw1, \
                 tc.tile_pool(name="b_w2", bufs=3) as bw2, \
                 tc.tile_pool(name="b_out", bufs=3) as bout, \
                 tc.tile_pool(name="b_st", bufs=4) as bst, \
                 tc.tile_pool(name="b_t2", bufs=4) as bt2, \
                 tc.tile_pool(name="b_scr", bufs=2) as bscr, \
                 tc.tile_pool(name="ps_tr", bufs=2, space="PSUM") as ps_tr, \
                 tc.tile_pool(name="ps_z", bufs=4, space="PSUM") as ps_z, \
                 tc.tile_pool(name="ps_y", bufs=2, space="PSUM") as ps_y:

                hT = bsb.tile([128, DC, TOK], dt.float32r)

                def emit_transposes(qlo_, qhi_):
                    # half-tile pipelining: 2 PSUM slots, evac alternates ACT/DVE
                    for qi in range(qlo_, qhi_):
                        for hf in range(2):
                            tr_ps = ps_tr.tile([128, 512], dt.float32r, tag="tr",
                                               name=f"tr_{qi}_{hf}")
                            for j in range(4):
                                dc = hf * 4 + j
                                nc.tensor.transpose(
                                    tr_ps[:, j * 128:(j + 1) * 128],
                                    h_tiles[qi][:, dc * 128:(dc + 1) * 128],
                                    ident_r[:])
                            dst = hT[:, hf * 4:(hf + 1) * 4,
                                     qi * 128:(qi + 1) * 128]
                            src = tr_ps[:].rearrange("p (c t) -> p c t", c=4)
                            if (qi + hf) % 2 == 0:
                                nc.scalar.copy(dst, src)
                            else:
                                nc.vector.tensor_copy(dst, src)

                for tg in range(NTG):
                    if tg == 0:
                        emit_transposes(0, 4)
                    t0 = tg * TG
                    nt = TG // 128  # 4 token tiles in this group
                    # ---- stage A: gT = gelu(w1^T @ hT + b1) ----
                    gT = bsb.tile([128, FC, TG], dt.float32r, tag="gT")
                    for fc4 in range(FC // 2):
                        w1_t = bw1.tile([128, DC, 256], dt.float32r, tag="w1")
                        nc.sync.dma_start(w1_t[:, 0:4, :],
                                          w1T[:, 0:4, fc4 * 256:(fc4 + 1) * 256])
                        nc.sync.dma_start(w1_t[:, 4:8, :],
                                          w1T[:, 4:8, fc4 * 256:(fc4 + 1) * 256])
                        for kk in range(2):
                            fc = fc4 * 2 + kk
                            y_ps = ps_y.tile([128, TG], dt.float32, tag="y")
                            for dc in range(DC):
                                nc.tensor.matmul(y_ps[:],
                                                 w1_t[:, dc, kk * 128:(kk + 1) * 128],
                                                 hT[:, dc, t0:t0 + TG],
                                                 start=(dc == 0), stop=(dc == DC - 1))
                            nc.scalar.activation(gT[:, fc, :], y_ps[:], AF.Gelu,
                                                 bias=b1_t[:, fc:fc + 1])
                        # spread tg1's transposes through tg0's stage A
                        if tg == 0 and fc4 % 4 == 3:
                            qi_ = 4 + fc4 // 4
                            emit_transposes(qi_, qi_ + 1)
                    # ---- stage B: z = gT^T @ w2 per D-half; assemble t2 = z + h ----
                    t2s = [bt2.tile([128, 2, 512], dt.float32, tag="t2",
                                    name=f"t2_{tg}_{i}")
                           for i in range(nt)]
                    bn64 = bst.tile([128, nt, 2, 6], dt.float32, tag="bn64")
                    bn24 = bst.tile([128, nt, 2], dt.float32, tag="bn24")
                    for dh in range(2):
                        dsl = slice(dh * 512, (dh + 1) * 512)
                        z_ps = [ps_z.tile([128, 512], dt.float32, tag="z",
                                          name=f"z_{tg}_{dh}_{i}")
                                for i in range(nt)]
                        for fc4 in range(FC // 4):
                            w2_t = bw2.tile([128, 4, 512], dt.float32r, tag="w2")
                            nc.sync.dma_start(w2_t[:],
                                              w2T[:, fc4 * 4:(fc4 + 1) * 4, dsl])
                            for kk in range(4):
                                fc = fc4 * 4 + kk
                                for tt in range(nt):
                                    nc.tensor.matmul(
                                        z_ps[tt][:],
                                        gT[:, fc, tt * 128:(tt + 1) * 128],
                                        w2_t[:, kk, :],
                                        start=(fc == 0), stop=(fc == FC - 1))
                        for tt in range(nt):
                            qi = tg * nt + tt
                            nc.vector.tensor_add(
                                t2s[tt][:, dh, :], z_ps[tt][:],
                                h_tiles[qi][:, dsl].bitcast(dt.float32))
                            if use_b2:
                                nc.vector.tensor_add(t2s[tt][:, dh, :],
                                                     t2s[tt][:, dh, :],
                                                     gb_tiles["b2b"][:, dsl])
                            nc.vector.bn_stats(bn64[:, tt, dh, :], t2s[tt][:, dh, :])
                    # ---- LN2 (stats vectorized over the nt tiles) + store ----
                    for tt in range(nt):
                        nc.vector.bn_aggr(bn24[:, tt, :], bn64[:, tt, :, :])
                    rstd4 = _emit_rsqrt(nc, bst, bn24[:, :, 1], magic_t, "b", w=nt)
                    nm4 = bst.tile([128, nt], dt.float32, tag="nm4")
                    nc.vector.scalar_tensor_tensor(nm4[:], bn24[:, :, 0], -1.0,
                                                   rstd4[:], op0=ALU.mult,
                                                   op1=ALU.mult)
                    for tt in range(nt):
                        qi = tg * nt + tt
                        t2v = t2s[tt][:].rearrange("p a b -> p (a b)")
                        o_sb = bout.tile([128, D], dt.float32, tag="osb")
                        if use_g2:
                            on = bscr.tile([128, D], dt.float32, tag="on")
                            nc.scalar.activation(on[:], t2v, AF.Identity,
                                                 bias=nm4[:, tt:tt + 1],
                                                 scale=rstd4[:, tt:tt + 1])
                            og = bscr.tile([128, D], dt.float32, tag="og")
                            nc.vector.tensor_mul(og[:], on[:], gb_tiles["g2b"][:])
                            nc.vector.tensor_add(o_sb[:], og[:], gb_tiles["b2lb"][:])
                        else:
                            nc.scalar.activation(o_sb[:], t2v, AF.Identity,
                                                 bias=nm4[:, tt:tt + 1],
                                                 scale=rstd4[:, tt:tt + 1])
                        nc.sync.dma_start(out[:, qi, :], o_sb[:])

    nc.finalize()
    return nc


_BUILD_CACHE = {}


def kernel(hidden_state, attention_mask, w1, b1, w2, b2,
           ln1_g, ln1_b, ln2_g, ln2_b) -> np.ndarray:
    hidden_state = np.asarray(hidden_state, dtype=np.float32)
    attention_mask = np.asarray(attention_mask, dtype=np.float32)
    w1 = np.asarray(w1, dtype=np.float32)
    b1 = np.asarray(b1, dtype=np.float32)
    w2 = np.asarray(w2, dtype=np.float32)
    b2 = np.asarray(b2, dtype=np.float32)
    ln1_g = np.asarray(ln1_g, dtype=np.float32)
    ln1_b = np.asarray(ln1_b, dtype=np.float32)
    ln2_g = np.asarray(ln2_g, dtype=np.float32)
    ln2_b = np.asarray(ln2_b, dtype=np.float32)

    use_g1 = not (np.all(ln1_g == 1.0) and np.all(ln1_b == 0.0))
    use_g2 = not (np.all(ln2_g == 1.0) and np.all(ln2_b == 0.0))
    use_b2 = bool(np.any(b2 != 0.0))

    key = (use_g1, use_g2, use_b2)
    if key not in _BUILD_CACHE:
        _BUILD_CACHE[key] = build(*key)
    nc = _BUILD_CACHE[key]

    # host-side layout prep (shared across cores of the same batch)
    w1_l = to_f32r(w1.reshape(DC, 128, DFF).transpose(1, 0, 2))     # [128, DC, DFF]
    w2_l = to_f32r(w2.reshape(FC, 128, D).transpose(1, 0, 2))       # [128, FC, D]
    b1_l = np.ascontiguousarray(b1.reshape(FC, 128).T)              # [128, FC]

    in_maps = []
    for c in range(N_CORES):
        b_idx, half = c // 2, c % 2
        x = hidden_state[b_idx]                                     # [S, D]
        xr = to_f32r(x)
        xT_l = np.ascontiguousarray(
            x.astype(ml_dtypes.bfloat16).T.reshape(DC, 128, S)
            .transpose(1, 0, 2))                                    # [128, DC, S] bf16
        xkd_f = xr.reshape(KC, 128, D).transpose(1, 0, 2)           # [128, KC, D] f32r
        # NOTE: xkd rows used for residual must be this core's token rows:
        # token tile qi (0..7) = batch row chunk half*8 + qi. The kernel indexes
        # xkd_t[:, qi, :] for the residual, so reorder chunks: put this core's
        # 8 chunks first, the other half after (AV sweeps all 16 either way,
        # with the exp/mask/pT ordering matching the chunk order).
        order = list(range(half * 8, half * 8 + 8)) + \
                list(range((1 - half) * 8, (1 - half) * 8 + 8))
        xkd_f = xkd_f[:, order, :]
        xres_l = np.ascontiguousarray(xkd_f[:, 0:NQ, :])            # this core's rows
        xkd_l = np.ascontiguousarray(
            x.astype(ml_dtypes.bfloat16).reshape(KC, 128, D)
            .transpose(1, 0, 2)[:, order, :])                       # [128, KC, D] bf16
        mask_l = np.ascontiguousarray(
            attention_mask[b_idx, 0].reshape(KC, 128).T)            # [128, KC]
        mask_l = np.ascontiguousarray(mask_l[:, order])
        # xT column order must match xkd chunk order for the scores (keys) too:
        kcols = np.concatenate([np.arange(o * 128, (o + 1) * 128) for o in order])
        xT_l = np.ascontiguousarray(xT_l[:, :, kcols])
        # queries: this core's tokens are the FIRST 8 chunks in reordered space
        im = dict(xT=xT_l, xkd=xkd_l, xres=xres_l, maskT=mask_l, w1T=w1_l,
          w2T=w2_l, b1T=b1_l)
        if use_g1:
            im["g1b"] = np.ascontiguousarray(np.broadcast_to(ln1_g, (128, D)))
            im["b1b"] = np.ascontiguousarray(np.broadcast_to(ln1_b, (128, D)))
        if use_g2:
            im["g2b"] = np.ascontiguousarray(np.broadcast_to(ln2_g, (128, D)))
            im["b2lb"] = np.ascontiguousarray(np.broadcast_to(ln2_b, (128, D)))
        if use_b2:
            im["b2b"] = np.ascontiguousarray(np.broadcast_to(b2, (128, D)))
        in_maps.append(im)

    res = run_bass_kernel_spmd(nc, in_maps, core_ids=list(range(N_CORES)))

    out_full = np.empty((B, S, D), dtype=np.float32)
    for c in range(N_CORES):
        b_idx, half = c // 2, c % 2
        o = res.results[c]["out"]                                   # [128, NQ, D]
        rows = o.transpose(1, 0, 2).reshape(TOK, D)
        out_full[b_idx, half * TOK:(half + 1) * TOK] = rows
    return out_full
